# revision 28
# baseline (speedup 1.0000x reference)
"""Fused multi-head self-attention (degenerate seq-len-1) + LayerNorm for TRN2.

Math: with sequence length 1, softmax over the single key is exactly 1.0, so
attention output == v.  The whole module collapses to

    out = LayerNorm((x @ W_v.T + b_v) @ W_proj.T + b_proj) * gamma + beta
        = LayerNorm(x @ C.T + bias) * gamma + beta

with C = W_proj @ W_v and bias = W_proj @ b_v + b_proj (both batch-independent,
folded on the host).  The device kernel is a single [1024,4096]x[4096,4096]
matmul per core (batch data-parallel over 8 cores) fused with LayerNorm.

Precision scheme: LayerNorm (with no bias, unit gamma in this problem) is
scale-invariant, so arbitrary global scales on x and C are free.  The 4096-dim
contraction is split K = KA*128 (bf16) + KOB*128 (fp8e4 via DoubleRow, 2x PE
throughput).  fp8e4 elementwise RMS quant error ~2.4%; the dot-product error
contribution is sqrt(2*KOB/32)*2.4%, kept under the 2e-2 harness gate with
margin.
"""

import os
import sys

import numpy as np

if "/opt/trn_rl_repo" not in sys.path:
    sys.path.insert(0, "/opt/trn_rl_repo")

import ml_dtypes

P = 128              # SBUF partitions
DIM = 4096
B = 8192
NCORES = 8
BL = B // NCORES     # batch rows per core
BT = BL // P         # b tiles per core
KO = DIM // P        # contraction tiles (32)
JC = 512             # moving free-dim chunk (output cols per matmul)
NJC = DIM // JC      # 8
EPS = 1e-5

# K split: KA 128-chunks in bf16, KOB in fp8e4 (DoubleRow pairs). KOB even.
KOB = int(os.environ.get("KERNEL_KOB", "10"))
KA = KO - KOB
SX8 = 32.0           # fp8 scale for x  (|x| < 5.7  -> *32  < 240)
SC8 = 1024.0         # fp8 scale for C  (|C| < 0.16 -> *1024 < 240)

_BUILD_CACHE = {}


def _build(apply_bias: bool, apply_affine: bool):
    key = (apply_bias, apply_affine, KOB)
    if key in _BUILD_CACHE:
        return _BUILD_CACHE[key]

    import concourse.mybir as mybir
    import concourse.tile as tile
    from concourse.tile import add_dep_helper
    from concourse import bacc

    bf16 = mybir.dt.bfloat16
    f16 = mybir.dt.float16
    f8 = mybir.dt.float8e4
    f32 = mybir.dt.float32

    nc = bacc.Bacc("TRN2", target_bir_lowering=False, debug=False,
                   num_devices=NCORES)

    xta_d = nc.declare_dram_parameter("xta", [BT, P, KA, P], bf16,
                                      isOutput=False)
    cta_d = nc.declare_dram_parameter("cta", [NJC, P, KA, JC], bf16,
                                      isOutput=False)
    if KOB:
        xtb_d = nc.declare_dram_parameter("xtb", [BT, P, KOB, P], f8,
                                          isOutput=False)
        ctb_d = nc.declare_dram_parameter("ctb", [NJC, P, KOB, JC], f8,
                                          isOutput=False)
    bias_d = nc.declare_dram_parameter("bias", [DIM], f32, isOutput=False)
    gamma_d = nc.declare_dram_parameter("gamma", [DIM], f32, isOutput=False)
    beta_d = nc.declare_dram_parameter("beta", [DIM], f32, isOutput=False)
    # fp16 output (upcast on host): halves the output traffic; 10 mantissa
    # bits is plenty for LayerNorm-scale values.
    out_d = nc.declare_dram_parameter("out", [BT, P, NJC, JC], f16,
                                      isOutput=True)

    # Non-bias builds apply the last two chunks straight from PSUM, so y_sb
    # holds NJC-2 chunks and the freed SBUF buys a third cta stream buffer
    # (which lets chunks 6/7 prefetch early enough for the bt-major tail).
    YC = NJC if apply_bias else NJC - 2
    WBUFS = 2 if apply_bias else 3

    with tile.TileContext(nc) as tc:
        with tc.tile_pool(name="xpool", bufs=1) as xpool, \
             tc.tile_pool(name="wpool", bufs=WBUFS) as wpool, \
             tc.tile_pool(name="ypool", bufs=1) as ypool, \
             tc.tile_pool(name="spool", bufs=1) as spool, \
             tc.tile_pool(name="opool", bufs=3) as opool, \
             tc.tile_pool(name="small", bufs=4) as small, \
             tc.tile_pool(name="ppool", bufs=4, space="PSUM") as ppool:

            xta_sb = xpool.tile([P, BT, KA, P], bf16, name="xta_sb")
            xtb_sb = (xpool.tile([P, BT, KOB, P], f8, name="xtb_sb")
                      if KOB else None)

            # PE warmup: independent matmuls on a memset scratch tile run
            # during the initial DMA head (no data deps), so the HAM clock
            # gate reaches 2.4 GHz before the real matmuls start.  ~16 cold
            # MMs cover the 3.4us SHORT window; the first chunk's data lands
            # at ~8-10us, so don't queue more than that (warmups would block
            # the real MMs behind them in the FIFO engine queue).
            warm_sb = small.tile([P, 384], bf16)
            nc.gpsimd.memset(warm_sb, 0.0)
            warm_ps = ppool.tile([P, 256], f32, name="warm_ps", tag="ps")
            for _ in range(60):
                nc.tensor.matmul(warm_ps, lhsT=warm_sb[:, 0:P],
                                 rhs=warm_sb[:, P:P + 256],
                                 start=True, stop=True)

            # Prefetch chunk 0 (both dtypes) split finely so the first
            # accumulation group's dependencies resolve early, then x b-tile
            # 0, chunk 1, then the remaining x b-tiles.
            ct_tiles = {}
            ct_tiles[0] = wpool.tile([P, KA, JC], bf16, name="cta_sb",
                                     tag="cta")
            ctb_tiles = {}
            if KOB:
                ctb_tiles[0] = wpool.tile([P, KOB, JC], f8, name="ctb_sb",
                                          tag="ctb")
            # HEAD: all hardware-dynamic DMA queues share one pool of 16
            # DMA engines (~330 GB/s aggregate), and each queue keeps ~6
            # instructions in flight, so bulk prefetch packets would
            # interleave with (and starve) the critical first-phase data.
            # Critical set = cta c0 + xta bt0 + fp8 c0/bt0 (~4 MiB), split
            # into ko-ordered pieces across all three rings; every other
            # input DMA is dependency-gated on the tail of the critical set.
            rings = [nc.sync, nc.scalar, nc.gpsimd]
            crit = []
            if KOB:
                crit.append(nc.gpsimd.dma_start(out=xtb_sb[:, 0],
                                                in_=xtb_d[0]))
                crit.append(nc.gpsimd.dma_start(out=ctb_tiles[0],
                                                in_=ctb_d[0]))
            cuts = list(range(0, KA - 2, 3))
            for i, (a, b) in enumerate(zip(cuts, cuts[1:] + [KA])):
                crit.append(rings[i % 3].dma_start(out=ct_tiles[0][:, a:b],
                                                   in_=cta_d[0, :, a:b]))
            h = KA // 2
            crit.append(nc.sync.dma_start(out=xta_sb[:, 0, :h],
                                          in_=xta_d[0, :, :h]))
            crit.append(nc.scalar.dma_start(out=xta_sb[:, 0, h:],
                                            in_=xta_d[0, :, h:]))
            gate = crit[-3].ins

            def gated(eng, out, in_):
                d = eng.dma_start(out=out, in_=in_)
                add_dep_helper(d.ins, gate,
                               reason="bulk input DMA after critical head")

            ct_tiles[1] = wpool.tile([P, KA, JC], bf16, name="cta_sb",
                                     tag="cta")
            if KOB:
                ctb_tiles[1] = wpool.tile([P, KOB, JC], f8, name="ctb_sb",
                                          tag="ctb")
            for bt in range(1, BT):
                gated(rings[bt % 3], xta_sb[:, bt], xta_d[bt])
                if KOB:
                    gated(rings[(bt + 1) % 3], xtb_sb[:, bt], xtb_d[bt])
            gated(nc.sync, ct_tiles[1], cta_d[1])
            if KOB:
                gated(nc.gpsimd, ctb_tiles[1], ctb_d[1])

            # y (pre-norm matmul result) stays resident in bf16.
            y_sb = ypool.tile([P, BT, YC, JC], bf16)
            # Per-chunk bn_stats, aggregated per b-tile at the end.
            stats_sb = spool.tile([P, BT, NJC, 6], f32)

            eps_sb = small.tile([P, 1], f32)
            nc.vector.memset(eps_sb, EPS)

            bias_sb = None
            if apply_bias:
                bias_sb = spool.tile([P, NJC, JC], f32)
                nc.sync.dma_start(out=bias_sb,
                                  in_=bias_d.ap().to_broadcast([P, NJC, JC]))

            gamma_sb = beta_sb = None
            if apply_affine:
                gamma_sb = spool.tile([P, NJC, JC], f32)
                nc.sync.dma_start(out=gamma_sb,
                                  in_=gamma_d.ap().to_broadcast([P, NJC, JC]))
                beta_sb = spool.tile([P, NJC, JC], f32)
                nc.sync.dma_start(out=beta_sb,
                                  in_=beta_d.ap().to_broadcast([P, NJC, JC]))

            JL = NJC // 2  # two LayerNorm-apply chunks of 2048 columns

            def layernorm_apply(bt, ps6=None, ps7=None):
                """Aggregate stats and write the normalized b-tile.

                The apply is (y - mu) * rstd, computed as two half-tiles:
                one on the Scalar engine (Identity(y*rstd + (-mu*rstd)) with
                per-partition scale/bias) and one on DVE (tensor_scalar), so
                each engine's per-b-tile work stays under the PE shadow of
                the remaining matmuls.  Chunks NJC-2/NJC-1 never went
                through PSUM->SBUF eviction; they're normalized straight out
                of PSUM (ps6/ps7).  Out-DMAs alternate between the gpsimd
                and sync rings.
                """
                mv = small.tile([P, 2], f32)
                nc.vector.bn_aggr(mv, stats_sb[:, bt, :, :])
                std = small.tile([P, 1], f32)
                nc.scalar.activation(std, mv[:, 1:2],
                                     mybir.ActivationFunctionType.Sqrt,
                                     bias=eps_sb)
                rstd = small.tile([P, 1], f32)
                nc.vector.reciprocal(rstd, std)
                nmr = small.tile([P, 1], f32)
                nc.vector.tensor_scalar(
                    nmr, mv[:, 0:1], scalar1=rstd, scalar2=-1.0,
                    op0=mybir.AluOpType.mult, op1=mybir.AluOpType.mult,
                )

                def dve_norm(dst, src):
                    nc.vector.tensor_scalar(
                        dst, src, scalar1=mv[:, 0:1], scalar2=rstd,
                        op0=mybir.AluOpType.subtract,
                        op1=mybir.AluOpType.mult,
                    )

                for i, j0 in enumerate(range(0, NJC, JL)):
                    o = opool.tile([P, JL, JC], f16)
                    if i == 0:
                        nc.scalar.activation(
                            o, y_sb[:, bt, j0:j0 + JL, :],
                            mybir.ActivationFunctionType.Identity,
                            bias=nmr, scale=rstd,
                        )
                    elif ps6 is not None:
                        dve_norm(o[:, :JL - 2, :],
                                 y_sb[:, bt, j0:j0 + JL - 2, :])
                        dve_norm(o[:, JL - 2, :], ps6)
                        dve_norm(o[:, JL - 1, :], ps7)
                    else:
                        dve_norm(o, y_sb[:, bt, j0:j0 + JL, :])
                    if apply_affine:
                        nc.vector.tensor_mul(o, o, gamma_sb[:, j0:j0 + JL, :])
                        nc.vector.tensor_add(o, o, beta_sb[:, j0:j0 + JL, :])
                    eng = nc.gpsimd if (bt + i) % 2 == 0 else nc.sync
                    eng.dma_start(out=out_d[bt, :, j0:j0 + JL, :], in_=o)

            def matmul_group(ct_sb, ctb_sb, jc, bt):
                ps = ppool.tile([P, JC], f32, name="ps", tag="ps")
                # Interleave each fp8 DoubleRow MM after a bf16 MM: a DR
                # LDWEIGHTS pulls ahead (hides) under a preceding bf16 MM
                # but only under every other DR MM, so back-to-back DR MMs
                # pay ~190ns of exposed 256-column weight load each pair.
                ndr = KOB // 2
                lead = KA - ndr
                for ko in range(KA):
                    nc.tensor.matmul(
                        ps,
                        lhsT=xta_sb[:, bt, ko, :],
                        rhs=ct_sb[:, ko, :],
                        start=(ko == 0),
                        stop=(ko == KA - 1 and KOB == 0),
                    )
                    i = ko - lead
                    if 0 <= i:
                        kp = 2 * i
                        nc.tensor.matmul(
                            ps,
                            lhsT=xtb_sb[:, bt, kp:kp + 2, :],
                            rhs=ctb_sb[:, kp:kp + 2, :],
                            start=False,
                            stop=(kp == KOB - 2),
                            perf_mode=mybir.MatmulPerfMode.DoubleRow,
                        )
                if apply_bias:
                    nc.vector.tensor_add(y_sb[:, bt, jc, :], ps,
                                         bias_sb[:, jc, :])
                    nc.vector.bn_stats(stats_sb[:, bt, jc, :],
                                       y_sb[:, bt, jc, :])
                else:
                    # DVE reads PSUM directly for the LayerNorm statistics;
                    # ACT evicts PSUM (cast to bf16) except for the last two
                    # chunks, which the epilogue normalizes out of PSUM.
                    if jc < YC:
                        nc.scalar.activation(
                            y_sb[:, bt, jc, :], ps,
                            mybir.ActivationFunctionType.Copy)
                    nc.vector.bn_stats(stats_sb[:, bt, jc, :], ps)
                return ps

            def prefetch(jc):
                if jc + 2 < NJC:
                    ct_tiles[jc + 2] = wpool.tile([P, KA, JC], bf16,
                                                  name="cta_sb", tag="cta")
                    nc.sync.dma_start(out=ct_tiles[jc + 2], in_=cta_d[jc + 2])
                    if KOB:
                        ctb_tiles[jc + 2] = wpool.tile([P, KOB, JC], f8,
                                                       name="ctb_sb",
                                                       tag="ctb")
                        nc.gpsimd.dma_start(out=ctb_tiles[jc + 2],
                                            in_=ctb_d[jc + 2])

            # Phases 0..NJC-3: jc-major (each chunk streams through all
            # b-tiles).  Last two phases: bt-major, so b-tile completions
            # (LayerNorm apply + 1 MiB output DMA each) spread over ~100us
            # of matmul shadow instead of piling up behind the last phase.
            for jc in range(NJC - 2):
                ct_sb = ct_tiles.pop(jc)
                ctb_sb = ctb_tiles.pop(jc) if KOB else None
                for bt in range(BT):
                    if bt == 4:
                        # Mid-phase prefetch keeps the chunk stream off the
                        # DMA engines during the critical head window.
                        prefetch(jc)
                    matmul_group(ct_sb, ctb_sb, jc, bt)
            c6, c7 = ct_tiles.pop(NJC - 2), ct_tiles.pop(NJC - 1)
            cb6 = ctb_tiles.pop(NJC - 2) if KOB else None
            cb7 = ctb_tiles.pop(NJC - 1) if KOB else None
            for bt in range(BT):
                ps6 = matmul_group(c6, cb6, NJC - 2, bt)
                ps7 = matmul_group(c7, cb7, NJC - 1, bt)
                if apply_bias:
                    layernorm_apply(bt)
                else:
                    layernorm_apply(bt, ps6=ps6, ps7=ps7)

    nc.compile()
    _BUILD_CACHE[key] = nc
    return nc


def kernel(x, W_qkv, b_qkv, W_proj, b_proj, gamma, beta):
    from concourse.bass_utils import run_bass_kernel_spmd

    x = np.asarray(x, dtype=np.float32)
    W_qkv = np.asarray(W_qkv, dtype=np.float32)
    b_qkv = np.asarray(b_qkv, dtype=np.float32)
    W_proj = np.asarray(W_proj, dtype=np.float32)
    b_proj = np.asarray(b_proj, dtype=np.float32)
    gamma = np.asarray(gamma, dtype=np.float32)
    beta = np.asarray(beta, dtype=np.float32)

    # Fold the two projections (q/k are dead: seq len 1 => attention == v).
    W_v = W_qkv[2 * DIM:3 * DIM, :]
    C = W_proj @ W_v                          # [j, k]
    bias_total = W_proj @ b_qkv[2 * DIM:] + b_proj
    if KOB:
        bias_total = bias_total * (SX8 * SC8)

    # C^T tiled for streaming: ct[jc, p, ko, jl] = C[jc*JC+jl, ko*P+p]
    # With a hybrid split, the fp8 partial sums carry the SX8*SC8 = 2^15
    # scale, so the bf16 part must carry it too (exact in bf16: power of 2).
    # LayerNorm is scale-invariant, so the scale never needs removing.
    scale_tot = SX8 * SC8 if KOB else 1.0
    CtT = C.T.reshape(KO, P, NJC, JC).transpose(2, 1, 0, 3)  # [NJC,P,KO,JC]
    Cta = np.ascontiguousarray(CtT[:, :, :KA] * scale_tot).astype(
        ml_dtypes.bfloat16)
    if KOB:
        Ctb = np.ascontiguousarray(
            np.clip(CtT[:, :, KA:] * SC8, -240, 240)).astype(
            ml_dtypes.float8_e4m3)

    apply_bias = bool(np.any(bias_total))
    apply_affine = not (np.all(gamma == 1.0) and np.all(beta == 0.0))

    nc = _build(apply_bias, apply_affine)

    in_maps = []
    for i in range(NCORES):
        xs = x[i * BL:(i + 1) * BL]           # [BL, DIM]
        # xt[bt, p, ko, b'] = xs[bt*P + b', ko*P + p]
        xt = xs.T.reshape(KO, P, BT, P).transpose(2, 1, 0, 3)  # [BT,P,KO,P]
        m = {
            "xta": np.ascontiguousarray(
                xt[:, :, :KA]).astype(ml_dtypes.bfloat16),
            "cta": Cta,
            "bias": bias_total,
            "gamma": gamma,
            "beta": beta,
        }
        if KOB:
            m["xtb"] = np.ascontiguousarray(
                np.clip(xt[:, :, KA:] * SX8, -240, 240)).astype(
                ml_dtypes.float8_e4m3)
            m["ctb"] = Ctb
        in_maps.append(m)

    trace = bool(int(os.environ.get("KERNEL_TRACE", "0")))
    res = run_bass_kernel_spmd(nc, in_maps, core_ids=list(range(NCORES)),
                               trace=trace)
    if trace:
        kernel.last_exec_time_ns = res.exec_time_ns
        kernel.last_results = res

    out = np.concatenate(
        [r["out"].reshape(BL, DIM).astype(np.float32) for r in res.results],
        axis=0,
    )
    return out


# revision 31
# speedup vs baseline: 1.0113x; 1.0113x over previous
"""Fused multi-head self-attention (degenerate seq-len-1) + LayerNorm for TRN2.

Math: with sequence length 1, softmax over the single key is exactly 1.0, so
attention output == v.  The whole module collapses to

    out = LayerNorm((x @ W_v.T + b_v) @ W_proj.T + b_proj) * gamma + beta
        = LayerNorm(x @ C.T + bias) * gamma + beta

with C = W_proj @ W_v and bias = W_proj @ b_v + b_proj (both batch-independent,
folded on the host).  The device kernel is a single [1024,4096]x[4096,4096]
matmul per core (batch data-parallel over 8 cores) fused with LayerNorm.

Precision scheme: LayerNorm (with no bias, unit gamma in this problem) is
scale-invariant, so arbitrary global scales on x and C are free.  The 4096-dim
contraction is split K = KA*128 (bf16) + KOB*128 (fp8e4 via DoubleRow, 2x PE
throughput).  fp8e4 elementwise RMS quant error ~2.4%; the dot-product error
contribution is sqrt(2*KOB/32)*2.4%, kept under the 2e-2 harness gate with
margin.
"""

import os
import sys

import numpy as np

if "/opt/trn_rl_repo" not in sys.path:
    sys.path.insert(0, "/opt/trn_rl_repo")

import ml_dtypes

P = 128              # SBUF partitions
DIM = 4096
B = 8192
NCORES = 8
BL = B // NCORES     # batch rows per core
BT = BL // P         # b tiles per core
KO = DIM // P        # contraction tiles (32)
JC = 512             # moving free-dim chunk (output cols per matmul)
NJC = DIM // JC      # 8
EPS = 1e-5

# K split: KA 128-chunks in bf16, KOB in fp8e4 (DoubleRow pairs). KOB even.
KOB = int(os.environ.get("KERNEL_KOB", "10"))
KA = KO - KOB
SX8 = 32.0           # fp8 scale for x  (|x| < 5.7  -> *32  < 240)
SC8 = 1024.0         # fp8 scale for C  (|C| < 0.16 -> *1024 < 240)

_BUILD_CACHE = {}


def _build(apply_bias: bool, apply_affine: bool):
    key = (apply_bias, apply_affine, KOB)
    if key in _BUILD_CACHE:
        return _BUILD_CACHE[key]

    import concourse.mybir as mybir
    import concourse.tile as tile
    from concourse.tile import add_dep_helper
    from concourse import bacc

    bf16 = mybir.dt.bfloat16
    f16 = mybir.dt.float16
    f8 = mybir.dt.float8e4
    f32 = mybir.dt.float32

    nc = bacc.Bacc("TRN2", target_bir_lowering=False, debug=False,
                   num_devices=NCORES)

    xta_d = nc.declare_dram_parameter("xta", [BT, P, KA, P], bf16,
                                      isOutput=False)
    cta_d = nc.declare_dram_parameter("cta", [NJC, P, KA, JC], bf16,
                                      isOutput=False)
    if KOB:
        xtb_d = nc.declare_dram_parameter("xtb", [BT, P, KOB, P], f8,
                                          isOutput=False)
        ctb_d = nc.declare_dram_parameter("ctb", [NJC, P, KOB, JC], f8,
                                          isOutput=False)
    bias_d = nc.declare_dram_parameter("bias", [DIM], f32, isOutput=False)
    gamma_d = nc.declare_dram_parameter("gamma", [DIM], f32, isOutput=False)
    beta_d = nc.declare_dram_parameter("beta", [DIM], f32, isOutput=False)
    # fp16 output (upcast on host): halves the output traffic; 10 mantissa
    # bits is plenty for LayerNorm-scale values.
    out_d = nc.declare_dram_parameter("out", [BT, P, NJC, JC], f16,
                                      isOutput=True)

    # Non-bias builds apply the last two chunks straight from PSUM, so y_sb
    # holds NJC-2 chunks and the freed SBUF buys a third cta stream buffer
    # (which lets chunks 6/7 prefetch early enough for the bt-major tail).
    YC = NJC if apply_bias else NJC - 2
    WBUFS = 2 if apply_bias else 3

    with tile.TileContext(nc) as tc:
        with tc.tile_pool(name="xpool", bufs=1) as xpool, \
             tc.tile_pool(name="wpool", bufs=WBUFS) as wpool, \
             tc.tile_pool(name="ypool", bufs=1) as ypool, \
             tc.tile_pool(name="spool", bufs=1) as spool, \
             tc.tile_pool(name="opool", bufs=3) as opool, \
             tc.tile_pool(name="small", bufs=4) as small, \
             tc.tile_pool(name="ppool", bufs=4, space="PSUM") as ppool:

            xta_sb = xpool.tile([P, BT, KA, P], bf16, name="xta_sb")
            xtb_sb = (xpool.tile([P, BT, KOB, P], f8, name="xtb_sb")
                      if KOB else None)

            # PE warmup: independent matmuls on a memset scratch tile run
            # during the initial DMA head (no data deps), so the HAM clock
            # gate reaches 2.4 GHz before the real matmuls start.  ~16 cold
            # MMs cover the 3.4us SHORT window; the first chunk's data lands
            # at ~8-10us, so don't queue more than that (warmups would block
            # the real MMs behind them in the FIFO engine queue).
            warm_sb = small.tile([P, 384], bf16)
            nc.gpsimd.memset(warm_sb, 0.0)
            warm_ps = ppool.tile([P, 256], f32, name="warm_ps", tag="ps")
            for _ in range(60):
                nc.tensor.matmul(warm_ps, lhsT=warm_sb[:, 0:P],
                                 rhs=warm_sb[:, P:P + 256],
                                 start=True, stop=True)

            # Prefetch chunk 0 (both dtypes) split finely so the first
            # accumulation group's dependencies resolve early, then x b-tile
            # 0, chunk 1, then the remaining x b-tiles.
            ct_tiles = {}
            ct_tiles[0] = wpool.tile([P, KA, JC], bf16, name="cta_sb",
                                     tag="cta")
            ctb_tiles = {}
            if KOB:
                ctb_tiles[0] = wpool.tile([P, KOB, JC], f8, name="ctb_sb",
                                          tag="ctb")
            # HEAD: all hardware-dynamic DMA queues share one pool of 16
            # DMA engines (~330 GB/s aggregate), and each queue keeps ~6
            # instructions in flight, so bulk prefetch packets would
            # interleave with (and starve) the critical first-phase data.
            # Critical set = cta c0 + xta bt0 + fp8 c0/bt0 (~4 MiB), split
            # into ko-ordered pieces across all three rings; every other
            # input DMA is dependency-gated on the tail of the critical set.
            rings = [nc.sync, nc.scalar, nc.gpsimd]
            crit = []
            # x b-tile 0 first on each ring (consumed from ko=0), then the
            # c0 pieces in ko order round-robin, fp8 bits on gpsimd.
            h = KA // 2
            crit.append(nc.sync.dma_start(out=xta_sb[:, 0, :h],
                                          in_=xta_d[0, :, :h]))
            crit.append(nc.scalar.dma_start(out=xta_sb[:, 0, h:],
                                            in_=xta_d[0, :, h:]))
            if KOB:
                crit.append(nc.gpsimd.dma_start(out=xtb_sb[:, 0],
                                                in_=xtb_d[0]))
                crit.append(nc.gpsimd.dma_start(out=ctb_tiles[0],
                                                in_=ctb_d[0]))
            cuts = list(range(0, KA - 2, 3))
            for i, (a, b) in enumerate(zip(cuts, cuts[1:] + [KA])):
                crit.append(rings[i % 3].dma_start(out=ct_tiles[0][:, a:b],
                                                   in_=cta_d[0, :, a:b]))
            gate = crit[-1].ins

            def gated(eng, out, in_):
                d = eng.dma_start(out=out, in_=in_)
                add_dep_helper(d.ins, gate,
                               reason="bulk input DMA after critical head")

            ct_tiles[1] = wpool.tile([P, KA, JC], bf16, name="cta_sb",
                                     tag="cta")
            if KOB:
                ctb_tiles[1] = wpool.tile([P, KOB, JC], f8, name="ctb_sb",
                                          tag="ctb")
            for bt in range(1, BT):
                gated(rings[bt % 3], xta_sb[:, bt], xta_d[bt])
                if KOB:
                    gated(rings[(bt + 1) % 3], xtb_sb[:, bt], xtb_d[bt])
            gated(nc.sync, ct_tiles[1], cta_d[1])
            if KOB:
                gated(nc.gpsimd, ctb_tiles[1], ctb_d[1])

            # y (pre-norm matmul result) stays resident in bf16.
            y_sb = ypool.tile([P, BT, YC, JC], bf16)
            # Per-chunk bn_stats, aggregated per b-tile at the end.
            stats_sb = spool.tile([P, BT, NJC, 6], f32)

            eps_sb = small.tile([P, 1], f32)
            nc.vector.memset(eps_sb, EPS)

            bias_sb = None
            if apply_bias:
                bias_sb = spool.tile([P, NJC, JC], f32)
                nc.sync.dma_start(out=bias_sb,
                                  in_=bias_d.ap().to_broadcast([P, NJC, JC]))

            gamma_sb = beta_sb = None
            if apply_affine:
                gamma_sb = spool.tile([P, NJC, JC], f32)
                nc.sync.dma_start(out=gamma_sb,
                                  in_=gamma_d.ap().to_broadcast([P, NJC, JC]))
                beta_sb = spool.tile([P, NJC, JC], f32)
                nc.sync.dma_start(out=beta_sb,
                                  in_=beta_d.ap().to_broadcast([P, NJC, JC]))

            JL = NJC // 2  # two LayerNorm-apply chunks of 2048 columns

            def layernorm_apply(bt, ps6=None, ps7=None):
                """Aggregate stats and write the normalized b-tile.

                The apply is (y - mu) * rstd, computed as two half-tiles:
                one on the Scalar engine (Identity(y*rstd + (-mu*rstd)) with
                per-partition scale/bias) and one on DVE (tensor_scalar), so
                each engine's per-b-tile work stays under the PE shadow of
                the remaining matmuls.  Chunks NJC-2/NJC-1 never went
                through PSUM->SBUF eviction; they're normalized straight out
                of PSUM (ps6/ps7).  Out-DMAs alternate between the gpsimd
                and sync rings.
                """
                mv = small.tile([P, 2], f32)
                nc.vector.bn_aggr(mv, stats_sb[:, bt, :, :])
                std = small.tile([P, 1], f32)
                nc.scalar.activation(std, mv[:, 1:2],
                                     mybir.ActivationFunctionType.Sqrt,
                                     bias=eps_sb)
                rstd = small.tile([P, 1], f32)
                nc.vector.reciprocal(rstd, std)
                nmr = small.tile([P, 1], f32)
                nc.vector.tensor_scalar(
                    nmr, mv[:, 0:1], scalar1=rstd, scalar2=-1.0,
                    op0=mybir.AluOpType.mult, op1=mybir.AluOpType.mult,
                )

                def dve_norm(dst, src):
                    nc.vector.tensor_scalar(
                        dst, src, scalar1=mv[:, 0:1], scalar2=rstd,
                        op0=mybir.AluOpType.subtract,
                        op1=mybir.AluOpType.mult,
                    )

                # Output DMAs are split 2 (+3 for the PSUM chunks) per
                # b-tile half and spread over the gpsimd+sync rings so the
                # final b-tile's 1 MiB drains in parallel pieces instead of
                # two serial 512 KiB transfers.
                orings = [nc.gpsimd, nc.sync]
                for i, j0 in enumerate(range(0, NJC, JL)):
                    o = opool.tile([P, JL, JC], f16)
                    pieces = [(0, JL)]
                    if i == 0:
                        nc.scalar.activation(
                            o, y_sb[:, bt, j0:j0 + JL, :],
                            mybir.ActivationFunctionType.Identity,
                            bias=nmr, scale=rstd,
                        )
                        pieces = [(0, 2), (2, JL)]
                    elif ps6 is not None:
                        dve_norm(o[:, :JL - 2, :],
                                 y_sb[:, bt, j0:j0 + JL - 2, :])
                        dve_norm(o[:, JL - 2, :], ps6)
                        dve_norm(o[:, JL - 1, :], ps7)
                        pieces = [(0, JL - 2), (JL - 2, JL - 1), (JL - 1, JL)]
                    else:
                        dve_norm(o, y_sb[:, bt, j0:j0 + JL, :])
                    if apply_affine:
                        nc.vector.tensor_mul(o, o, gamma_sb[:, j0:j0 + JL, :])
                        nc.vector.tensor_add(o, o, beta_sb[:, j0:j0 + JL, :])
                        pieces = [(0, JL)]
                    for k, (a, b) in enumerate(pieces):
                        eng = orings[(bt + i + k) % 2]
                        eng.dma_start(out=out_d[bt, :, j0 + a:j0 + b, :],
                                      in_=o[:, a:b, :])

            def matmul_group(ct_sb, ctb_sb, jc, bt):
                ps = ppool.tile([P, JC], f32, name="ps", tag="ps")
                # The fp8 DoubleRow MMs run as one streak after the bf16
                # MMs: measured cadence 28, 403, 216, 216, ... — only the
                # second DR pays an exposed weight load, so a streak is at
                # the ideal 2x DR rate (interleaving with bf16 is worse:
                # every bf16-after-DR then pays ~445ns).
                for ko in range(KA):
                    nc.tensor.matmul(
                        ps,
                        lhsT=xta_sb[:, bt, ko, :],
                        rhs=ct_sb[:, ko, :],
                        start=(ko == 0),
                        stop=(ko == KA - 1 and KOB == 0),
                    )
                for kp in range(0, KOB, 2):
                    nc.tensor.matmul(
                        ps,
                        lhsT=xtb_sb[:, bt, kp:kp + 2, :],
                        rhs=ctb_sb[:, kp:kp + 2, :],
                        start=False,
                        stop=(kp == KOB - 2),
                        perf_mode=mybir.MatmulPerfMode.DoubleRow,
                    )
                if apply_bias:
                    nc.vector.tensor_add(y_sb[:, bt, jc, :], ps,
                                         bias_sb[:, jc, :])
                    nc.vector.bn_stats(stats_sb[:, bt, jc, :],
                                       y_sb[:, bt, jc, :])
                else:
                    # DVE reads PSUM directly for the LayerNorm statistics;
                    # ACT evicts PSUM (cast to bf16) except for the last two
                    # chunks, which the epilogue normalizes out of PSUM.
                    if jc < YC:
                        nc.scalar.activation(
                            y_sb[:, bt, jc, :], ps,
                            mybir.ActivationFunctionType.Copy)
                    nc.vector.bn_stats(stats_sb[:, bt, jc, :], ps)
                return ps

            def prefetch(jc):
                if jc + 2 < NJC:
                    ct_tiles[jc + 2] = wpool.tile([P, KA, JC], bf16,
                                                  name="cta_sb", tag="cta")
                    nc.sync.dma_start(out=ct_tiles[jc + 2], in_=cta_d[jc + 2])
                    if KOB:
                        ctb_tiles[jc + 2] = wpool.tile([P, KOB, JC], f8,
                                                       name="ctb_sb",
                                                       tag="ctb")
                        nc.gpsimd.dma_start(out=ctb_tiles[jc + 2],
                                            in_=ctb_d[jc + 2])

            # Phases 0..NJC-3: jc-major (each chunk streams through all
            # b-tiles).  Last two phases: bt-major, so b-tile completions
            # (LayerNorm apply + 1 MiB output DMA each) spread over ~100us
            # of matmul shadow instead of piling up behind the last phase.
            for jc in range(NJC - 2):
                ct_sb = ct_tiles.pop(jc)
                ctb_sb = ctb_tiles.pop(jc) if KOB else None
                for bt in range(BT):
                    if bt == 4:
                        # Mid-phase prefetch keeps the chunk stream off the
                        # DMA engines during the critical head window.
                        prefetch(jc)
                    matmul_group(ct_sb, ctb_sb, jc, bt)
            c6, c7 = ct_tiles.pop(NJC - 2), ct_tiles.pop(NJC - 1)
            cb6 = ctb_tiles.pop(NJC - 2) if KOB else None
            cb7 = ctb_tiles.pop(NJC - 1) if KOB else None
            for bt in range(BT):
                ps6 = matmul_group(c6, cb6, NJC - 2, bt)
                ps7 = matmul_group(c7, cb7, NJC - 1, bt)
                if apply_bias:
                    layernorm_apply(bt)
                else:
                    layernorm_apply(bt, ps6=ps6, ps7=ps7)

    nc.compile()
    _BUILD_CACHE[key] = nc
    return nc


def kernel(x, W_qkv, b_qkv, W_proj, b_proj, gamma, beta):
    from concourse.bass_utils import run_bass_kernel_spmd

    x = np.asarray(x, dtype=np.float32)
    W_qkv = np.asarray(W_qkv, dtype=np.float32)
    b_qkv = np.asarray(b_qkv, dtype=np.float32)
    W_proj = np.asarray(W_proj, dtype=np.float32)
    b_proj = np.asarray(b_proj, dtype=np.float32)
    gamma = np.asarray(gamma, dtype=np.float32)
    beta = np.asarray(beta, dtype=np.float32)

    # Fold the two projections (q/k are dead: seq len 1 => attention == v).
    W_v = W_qkv[2 * DIM:3 * DIM, :]
    C = W_proj @ W_v                          # [j, k]
    bias_total = W_proj @ b_qkv[2 * DIM:] + b_proj
    if KOB:
        bias_total = bias_total * (SX8 * SC8)

    # C^T tiled for streaming: ct[jc, p, ko, jl] = C[jc*JC+jl, ko*P+p]
    # With a hybrid split, the fp8 partial sums carry the SX8*SC8 = 2^15
    # scale, so the bf16 part must carry it too (exact in bf16: power of 2).
    # LayerNorm is scale-invariant, so the scale never needs removing.
    scale_tot = SX8 * SC8 if KOB else 1.0
    CtT = C.T.reshape(KO, P, NJC, JC).transpose(2, 1, 0, 3)  # [NJC,P,KO,JC]
    Cta = np.ascontiguousarray(CtT[:, :, :KA] * scale_tot).astype(
        ml_dtypes.bfloat16)
    if KOB:
        Ctb = np.ascontiguousarray(
            np.clip(CtT[:, :, KA:] * SC8, -240, 240)).astype(
            ml_dtypes.float8_e4m3)

    apply_bias = bool(np.any(bias_total))
    apply_affine = not (np.all(gamma == 1.0) and np.all(beta == 0.0))

    nc = _build(apply_bias, apply_affine)

    in_maps = []
    for i in range(NCORES):
        xs = x[i * BL:(i + 1) * BL]           # [BL, DIM]
        # xt[bt, p, ko, b'] = xs[bt*P + b', ko*P + p]
        xt = xs.T.reshape(KO, P, BT, P).transpose(2, 1, 0, 3)  # [BT,P,KO,P]
        m = {
            "xta": np.ascontiguousarray(
                xt[:, :, :KA]).astype(ml_dtypes.bfloat16),
            "cta": Cta,
            "bias": bias_total,
            "gamma": gamma,
            "beta": beta,
        }
        if KOB:
            m["xtb"] = np.ascontiguousarray(
                np.clip(xt[:, :, KA:] * SX8, -240, 240)).astype(
                ml_dtypes.float8_e4m3)
            m["ctb"] = Ctb
        in_maps.append(m)

    trace = bool(int(os.environ.get("KERNEL_TRACE", "0")))
    res = run_bass_kernel_spmd(nc, in_maps, core_ids=list(range(NCORES)),
                               trace=trace)
    if trace:
        kernel.last_exec_time_ns = res.exec_time_ns
        kernel.last_results = res

    out = np.concatenate(
        [r["out"].reshape(BL, DIM).astype(np.float32) for r in res.results],
        axis=0,
    )
    return out


# revision 36
# speedup vs baseline: 1.0155x; 1.0041x over previous
"""Fused multi-head self-attention (degenerate seq-len-1) + LayerNorm for TRN2.

Math: with sequence length 1, softmax over the single key is exactly 1.0, so
attention output == v.  The whole module collapses to

    out = LayerNorm((x @ W_v.T + b_v) @ W_proj.T + b_proj) * gamma + beta
        = LayerNorm(x @ C.T + bias) * gamma + beta

with C = W_proj @ W_v and bias = W_proj @ b_v + b_proj (both batch-independent,
folded on the host).  The device kernel is a single [1024,4096]x[4096,4096]
matmul per core (batch data-parallel over 8 cores) fused with LayerNorm.

Precision scheme: LayerNorm (with no bias, unit gamma in this problem) is
scale-invariant, so arbitrary global scales on x and C are free.  The 4096-dim
contraction is split K = KA*128 (bf16) + KOB*128 (fp8e4 via DoubleRow, 2x PE
throughput).  fp8e4 elementwise RMS quant error ~2.4%; the dot-product error
contribution is sqrt(2*KOB/32)*2.4%, kept under the 2e-2 harness gate with
margin.
"""

import os
import sys

import numpy as np

if "/opt/trn_rl_repo" not in sys.path:
    sys.path.insert(0, "/opt/trn_rl_repo")

import ml_dtypes

P = 128              # SBUF partitions
DIM = 4096
B = 8192
NCORES = 8
BL = B // NCORES     # batch rows per core
BT = BL // P         # b tiles per core
KO = DIM // P        # contraction tiles (32)
JC = 512             # moving free-dim chunk (output cols per matmul)
NJC = DIM // JC      # 8
EPS = 1e-5

# K split: KA 128-chunks in bf16, KOB in fp8e4 (DoubleRow pairs). KOB even.
KOB = int(os.environ.get("KERNEL_KOB", "10"))
KA = KO - KOB
SX8 = 32.0           # fp8 scale for x  (|x| < 5.7  -> *32  < 240)
SC8 = 1024.0         # fp8 scale for C  (|C| < 0.16 -> *1024 < 240)

_BUILD_CACHE = {}


def _build(apply_bias: bool, apply_affine: bool):
    key = (apply_bias, apply_affine, KOB)
    if key in _BUILD_CACHE:
        return _BUILD_CACHE[key]

    import concourse.mybir as mybir
    import concourse.tile as tile
    from concourse.tile import add_dep_helper
    from concourse import bacc

    bf16 = mybir.dt.bfloat16
    f16 = mybir.dt.float16
    f8 = mybir.dt.float8e4
    f32 = mybir.dt.float32

    nc = bacc.Bacc("TRN2", target_bir_lowering=False, debug=False,
                   num_devices=NCORES)

    xta_d = nc.declare_dram_parameter("xta", [BT, P, KA, P], bf16,
                                      isOutput=False)
    cta_d = nc.declare_dram_parameter("cta", [NJC, P, KA, JC], bf16,
                                      isOutput=False)
    if KOB:
        xtb_d = nc.declare_dram_parameter("xtb", [BT, P, KOB, P], f8,
                                          isOutput=False)
        ctb_d = nc.declare_dram_parameter("ctb", [NJC, P, KOB, JC], f8,
                                          isOutput=False)
    bias_d = nc.declare_dram_parameter("bias", [DIM], f32, isOutput=False)
    gamma_d = nc.declare_dram_parameter("gamma", [DIM], f32, isOutput=False)
    beta_d = nc.declare_dram_parameter("beta", [DIM], f32, isOutput=False)
    # fp16 output (upcast on host): halves the output traffic; 10 mantissa
    # bits is plenty for LayerNorm-scale values.
    out_d = nc.declare_dram_parameter("out", [BT, P, NJC, JC], f16,
                                      isOutput=True)

    # Non-bias builds apply the last two chunks straight from PSUM, so y_sb
    # holds NJC-2 chunks and the freed SBUF buys a third cta stream buffer
    # (which lets chunks 6/7 prefetch early enough for the bt-major tail).
    YC = NJC if apply_bias else NJC - 2
    WBUFS = 2 if apply_bias else 3

    with tile.TileContext(nc) as tc:
        with tc.tile_pool(name="xpool", bufs=1) as xpool, \
             tc.tile_pool(name="wpool", bufs=WBUFS) as wpool, \
             tc.tile_pool(name="ypool", bufs=1) as ypool, \
             tc.tile_pool(name="spool", bufs=1) as spool, \
             tc.tile_pool(name="opool", bufs=3) as opool, \
             tc.tile_pool(name="small", bufs=4) as small, \
             tc.tile_pool(name="ppool", bufs=4, space="PSUM") as ppool:

            xta_sb = xpool.tile([P, BT, KA, P], bf16, name="xta_sb")
            xtb_sb = (xpool.tile([P, BT, KOB, P], f8, name="xtb_sb")
                      if KOB else None)

            # PE warmup: independent matmuls on a memset scratch tile run
            # during the initial DMA head (no data deps), so the HAM clock
            # gate reaches 2.4 GHz before the real matmuls start.  ~16 cold
            # MMs cover the 3.4us SHORT window; the first chunk's data lands
            # at ~8-10us, so don't queue more than that (warmups would block
            # the real MMs behind them in the FIFO engine queue).
            warm_sb = small.tile([P, 384], bf16)
            nc.gpsimd.memset(warm_sb, 0.0)
            warm_ps = ppool.tile([P, 256], f32, name="warm_ps", tag="ps")
            for _ in range(20):
                nc.tensor.matmul(warm_ps, lhsT=warm_sb[:, 0:P],
                                 rhs=warm_sb[:, P:P + 256],
                                 start=True, stop=True)

            # Prefetch chunk 0 (both dtypes) split finely so the first
            # accumulation group's dependencies resolve early, then x b-tile
            # 0, chunk 1, then the remaining x b-tiles.
            ct_tiles = {}
            ct_tiles[0] = wpool.tile([P, KA, JC], bf16, name="cta_sb",
                                     tag="cta")
            ctb_tiles = {}
            if KOB:
                ctb_tiles[0] = wpool.tile([P, KOB, JC], f8, name="ctb_sb",
                                          tag="ctb")
            # HEAD: all hardware-dynamic DMA queues share one pool of 16
            # DMA engines (~330 GB/s aggregate), and each queue keeps ~6
            # instructions in flight, so bulk prefetch packets would
            # interleave with (and starve) the critical first-phase data.
            # Critical set = cta c0 + xta bt0 + fp8 c0/bt0 (~4 MiB), split
            # into ko-ordered pieces across all three rings; every other
            # input DMA is dependency-gated on the tail of the critical set.
            rings = [nc.sync, nc.scalar, nc.gpsimd]
            crit = []
            # x b-tile 0 first on each ring (consumed from ko=0), then the
            # c0 pieces in ko order round-robin, fp8 bits on gpsimd.
            h = KA // 2
            crit.append(nc.sync.dma_start(out=xta_sb[:, 0, :h],
                                          in_=xta_d[0, :, :h]))
            crit.append(nc.scalar.dma_start(out=xta_sb[:, 0, h:],
                                            in_=xta_d[0, :, h:]))
            if KOB:
                crit.append(nc.gpsimd.dma_start(out=xtb_sb[:, 0],
                                                in_=xtb_d[0]))
                crit.append(nc.gpsimd.dma_start(out=ctb_tiles[0],
                                                in_=ctb_d[0]))
            cuts = list(range(0, KA - 2, 3))
            for i, (a, b) in enumerate(zip(cuts, cuts[1:] + [KA])):
                crit.append(rings[i % 3].dma_start(out=ct_tiles[0][:, a:b],
                                                   in_=cta_d[0, :, a:b]))
            gate = crit[-1].ins

            def gated(eng, out, in_):
                d = eng.dma_start(out=out, in_=in_)
                add_dep_helper(d.ins, gate,
                               reason="bulk input DMA after critical head")

            # Ungated second tier: x b-tiles 1-3 and chunk 1 — needed inside
            # the first two (half-)phases, so they queue right behind the
            # critical set.  Everything later is gated.
            ct_tiles[1] = wpool.tile([P, KA, JC], bf16, name="cta_sb",
                                     tag="cta")
            if KOB:
                ctb_tiles[1] = wpool.tile([P, KOB, JC], f8, name="ctb_sb",
                                          tag="ctb")
            for bt in range(1, 4):
                rings[bt % 3].dma_start(out=xta_sb[:, bt], in_=xta_d[bt])
                if KOB:
                    rings[(bt + 1) % 3].dma_start(out=xtb_sb[:, bt],
                                                  in_=xtb_d[bt])
            nc.sync.dma_start(out=ct_tiles[1], in_=cta_d[1])
            if KOB:
                nc.gpsimd.dma_start(out=ctb_tiles[1], in_=ctb_d[1])
            for bt in range(4, BT):
                gated(rings[bt % 3], xta_sb[:, bt], xta_d[bt])
                if KOB:
                    gated(rings[(bt + 1) % 3], xtb_sb[:, bt], xtb_d[bt])

            # y (pre-norm matmul result) stays resident in bf16.
            y_sb = ypool.tile([P, BT, YC, JC], bf16)
            # Per-chunk bn_stats, aggregated per b-tile at the end.
            stats_sb = spool.tile([P, BT, NJC, 6], f32)

            eps_sb = small.tile([P, 1], f32)
            nc.vector.memset(eps_sb, EPS)

            bias_sb = None
            if apply_bias:
                bias_sb = spool.tile([P, NJC, JC], f32)
                nc.sync.dma_start(out=bias_sb,
                                  in_=bias_d.ap().to_broadcast([P, NJC, JC]))

            gamma_sb = beta_sb = None
            if apply_affine:
                gamma_sb = spool.tile([P, NJC, JC], f32)
                nc.sync.dma_start(out=gamma_sb,
                                  in_=gamma_d.ap().to_broadcast([P, NJC, JC]))
                beta_sb = spool.tile([P, NJC, JC], f32)
                nc.sync.dma_start(out=beta_sb,
                                  in_=beta_d.ap().to_broadcast([P, NJC, JC]))

            JL = NJC // 2  # two LayerNorm-apply chunks of 2048 columns

            def layernorm_apply(bt, ps6=None, ps7=None):
                """Aggregate stats and write the normalized b-tile.

                The apply is (y - mu) * rstd, computed as two half-tiles:
                one on the Scalar engine (Identity(y*rstd + (-mu*rstd)) with
                per-partition scale/bias) and one on DVE (tensor_scalar), so
                each engine's per-b-tile work stays under the PE shadow of
                the remaining matmuls.  Chunks NJC-2/NJC-1 never went
                through PSUM->SBUF eviction; they're normalized straight out
                of PSUM (ps6/ps7).  Out-DMAs alternate between the gpsimd
                and sync rings.
                """
                mv = small.tile([P, 2], f32)
                nc.vector.bn_aggr(mv, stats_sb[:, bt, :, :])
                std = small.tile([P, 1], f32)
                nc.scalar.activation(std, mv[:, 1:2],
                                     mybir.ActivationFunctionType.Sqrt,
                                     bias=eps_sb)
                rstd = small.tile([P, 1], f32)
                nc.vector.reciprocal(rstd, std)
                nmr = small.tile([P, 1], f32)
                nc.vector.tensor_scalar(
                    nmr, mv[:, 0:1], scalar1=rstd, scalar2=-1.0,
                    op0=mybir.AluOpType.mult, op1=mybir.AluOpType.mult,
                )

                def dve_norm(dst, src):
                    nc.vector.tensor_scalar(
                        dst, src, scalar1=mv[:, 0:1], scalar2=rstd,
                        op0=mybir.AluOpType.subtract,
                        op1=mybir.AluOpType.mult,
                    )

                # Output DMAs are split 2 (+3 for the PSUM chunks) per
                # b-tile half and spread over the gpsimd+sync rings so the
                # final b-tile's 1 MiB drains in parallel pieces instead of
                # two serial 512 KiB transfers.
                orings = [nc.gpsimd, nc.sync]
                for i, j0 in enumerate(range(0, NJC, JL)):
                    o = opool.tile([P, JL, JC], f16)
                    pieces = [(0, JL)]
                    if i == 0:
                        # Two ACT instructions so the first piece's output
                        # DMA can launch ~1us earlier.
                        nc.scalar.activation(
                            o[:, 0:2, :], y_sb[:, bt, j0:j0 + 2, :],
                            mybir.ActivationFunctionType.Identity,
                            bias=nmr, scale=rstd,
                        )
                        nc.scalar.activation(
                            o[:, 2:JL, :], y_sb[:, bt, j0 + 2:j0 + JL, :],
                            mybir.ActivationFunctionType.Identity,
                            bias=nmr, scale=rstd,
                        )
                        pieces = [(0, 2), (2, JL)]
                    elif ps6 is not None:
                        dve_norm(o[:, :JL - 2, :],
                                 y_sb[:, bt, j0:j0 + JL - 2, :])
                        dve_norm(o[:, JL - 2, :], ps6)
                        dve_norm(o[:, JL - 1, :], ps7)
                        pieces = [(0, JL - 2), (JL - 2, JL - 1), (JL - 1, JL)]
                    else:
                        dve_norm(o, y_sb[:, bt, j0:j0 + JL, :])
                    if apply_affine:
                        nc.vector.tensor_mul(o, o, gamma_sb[:, j0:j0 + JL, :])
                        nc.vector.tensor_add(o, o, beta_sb[:, j0:j0 + JL, :])
                        pieces = [(0, JL)]
                    for k, (a, b) in enumerate(pieces):
                        eng = orings[(bt + i + k) % 2]
                        eng.dma_start(out=out_d[bt, :, j0 + a:j0 + b, :],
                                      in_=o[:, a:b, :])

            def matmul_group(ct_sb, ctb_sb, jc, bt):
                ps = ppool.tile([P, JC], f32, name="ps", tag="ps")
                # The fp8 DoubleRow MMs run as one streak after the bf16
                # MMs: measured cadence 28, 403, 216, 216, ... — only the
                # second DR pays an exposed weight load, so a streak is at
                # the ideal 2x DR rate (interleaving with bf16 is worse:
                # every bf16-after-DR then pays ~445ns).
                for ko in range(KA):
                    nc.tensor.matmul(
                        ps,
                        lhsT=xta_sb[:, bt, ko, :],
                        rhs=ct_sb[:, ko, :],
                        start=(ko == 0),
                        stop=(ko == KA - 1 and KOB == 0),
                    )
                for kp in range(0, KOB, 2):
                    nc.tensor.matmul(
                        ps,
                        lhsT=xtb_sb[:, bt, kp:kp + 2, :],
                        rhs=ctb_sb[:, kp:kp + 2, :],
                        start=False,
                        stop=(kp == KOB - 2),
                        perf_mode=mybir.MatmulPerfMode.DoubleRow,
                    )
                if apply_bias:
                    nc.vector.tensor_add(y_sb[:, bt, jc, :], ps,
                                         bias_sb[:, jc, :])
                    nc.vector.bn_stats(stats_sb[:, bt, jc, :],
                                       y_sb[:, bt, jc, :])
                else:
                    # DVE reads PSUM directly for the LayerNorm statistics;
                    # ACT evicts PSUM (cast to bf16) except for the last two
                    # chunks, which the epilogue normalizes out of PSUM.
                    if jc < YC:
                        nc.scalar.activation(
                            y_sb[:, bt, jc, :], ps,
                            mybir.ActivationFunctionType.Copy)
                    nc.vector.bn_stats(stats_sb[:, bt, jc, :], ps)
                return ps

            def prefetch(c):
                ct_tiles[c] = wpool.tile([P, KA, JC], bf16,
                                         name="cta_sb", tag="cta")
                nc.sync.dma_start(out=ct_tiles[c], in_=cta_d[c])
                if KOB:
                    ctb_tiles[c] = wpool.tile([P, KOB, JC], f8,
                                              name="ctb_sb", tag="ctb")
                    nc.gpsimd.dma_start(out=ctb_tiles[c], in_=ctb_d[c])

            # Phases 0..NJC-3: jc-major (each chunk streams through all
            # b-tiles).  Last two phases: bt-major, so b-tile completions
            # (LayerNorm apply + 1 MiB output DMA each) spread over ~100us
            # of matmul shadow instead of piling up behind the last phase.
            # The first two chunks run as HALF-phases (4 b-tiles each) so
            # the start-of-kernel DMA demand (chunk + its b-tiles' x) is
            # halved: the head becomes compute-bound at ~10us instead of
            # data-walking for ~30us.  Chunk c is prefetched at phase
            # index c (its wpool slot is free by then; lands 2 phases
            # before use).
            sched = [(0, range(0, 4)), (1, range(0, 4)),
                     (0, range(4, 8)), (1, range(4, 8))]
            sched += [(jc, range(BT)) for jc in range(2, NJC - 2)]
            for pi, (jc, bts) in enumerate(sched):
                ct_sb = ct_tiles[jc]
                ctb_sb = ctb_tiles[jc] if KOB else None
                for gi, bt in enumerate(bts):
                    if gi == 1 and 2 <= pi < NJC:
                        prefetch(pi)
                    matmul_group(ct_sb, ctb_sb, jc, bt)
            c6, c7 = ct_tiles.pop(NJC - 2), ct_tiles.pop(NJC - 1)
            cb6 = ctb_tiles.pop(NJC - 2) if KOB else None
            cb7 = ctb_tiles.pop(NJC - 1) if KOB else None
            for bt in range(BT):
                ps6 = matmul_group(c6, cb6, NJC - 2, bt)
                ps7 = matmul_group(c7, cb7, NJC - 1, bt)
                if apply_bias:
                    layernorm_apply(bt)
                else:
                    layernorm_apply(bt, ps6=ps6, ps7=ps7)

    nc.compile()
    _BUILD_CACHE[key] = nc
    return nc


def kernel(x, W_qkv, b_qkv, W_proj, b_proj, gamma, beta):
    from concourse.bass_utils import run_bass_kernel_spmd

    x = np.asarray(x, dtype=np.float32)
    W_qkv = np.asarray(W_qkv, dtype=np.float32)
    b_qkv = np.asarray(b_qkv, dtype=np.float32)
    W_proj = np.asarray(W_proj, dtype=np.float32)
    b_proj = np.asarray(b_proj, dtype=np.float32)
    gamma = np.asarray(gamma, dtype=np.float32)
    beta = np.asarray(beta, dtype=np.float32)

    # Fold the two projections (q/k are dead: seq len 1 => attention == v).
    W_v = W_qkv[2 * DIM:3 * DIM, :]
    C = W_proj @ W_v                          # [j, k]
    bias_total = W_proj @ b_qkv[2 * DIM:] + b_proj
    if KOB:
        bias_total = bias_total * (SX8 * SC8)

    # C^T tiled for streaming: ct[jc, p, ko, jl] = C[jc*JC+jl, ko*P+p]
    # With a hybrid split, the fp8 partial sums carry the SX8*SC8 = 2^15
    # scale, so the bf16 part must carry it too (exact in bf16: power of 2).
    # LayerNorm is scale-invariant, so the scale never needs removing.
    scale_tot = SX8 * SC8 if KOB else 1.0
    CtT = C.T.reshape(KO, P, NJC, JC).transpose(2, 1, 0, 3)  # [NJC,P,KO,JC]
    Cta = np.ascontiguousarray(CtT[:, :, :KA] * scale_tot).astype(
        ml_dtypes.bfloat16)
    if KOB:
        Ctb = np.ascontiguousarray(
            np.clip(CtT[:, :, KA:] * SC8, -240, 240)).astype(
            ml_dtypes.float8_e4m3)

    apply_bias = bool(np.any(bias_total))
    apply_affine = not (np.all(gamma == 1.0) and np.all(beta == 0.0))

    nc = _build(apply_bias, apply_affine)

    in_maps = []
    for i in range(NCORES):
        xs = x[i * BL:(i + 1) * BL]           # [BL, DIM]
        # xt[bt, p, ko, b'] = xs[bt*P + b', ko*P + p]
        xt = xs.T.reshape(KO, P, BT, P).transpose(2, 1, 0, 3)  # [BT,P,KO,P]
        m = {
            "xta": np.ascontiguousarray(
                xt[:, :, :KA]).astype(ml_dtypes.bfloat16),
            "cta": Cta,
            "bias": bias_total,
            "gamma": gamma,
            "beta": beta,
        }
        if KOB:
            m["xtb"] = np.ascontiguousarray(
                np.clip(xt[:, :, KA:] * SX8, -240, 240)).astype(
                ml_dtypes.float8_e4m3)
            m["ctb"] = Ctb
        in_maps.append(m)

    trace = bool(int(os.environ.get("KERNEL_TRACE", "0")))
    res = run_bass_kernel_spmd(nc, in_maps, core_ids=list(range(NCORES)),
                               trace=trace)
    if trace:
        kernel.last_exec_time_ns = res.exec_time_ns
        kernel.last_results = res

    out = np.concatenate(
        [r["out"].reshape(BL, DIM).astype(np.float32) for r in res.results],
        axis=0,
    )
    return out


# revision 38
# speedup vs baseline: 1.0348x; 1.0190x over previous
"""Fused multi-head self-attention (degenerate seq-len-1) + LayerNorm for TRN2.

Math: with sequence length 1, softmax over the single key is exactly 1.0, so
attention output == v.  The whole module collapses to

    out = LayerNorm((x @ W_v.T + b_v) @ W_proj.T + b_proj) * gamma + beta
        = LayerNorm(x @ C.T + bias) * gamma + beta

with C = W_proj @ W_v and bias = W_proj @ b_v + b_proj (both batch-independent,
folded on the host).  The device kernel is a single [1024,4096]x[4096,4096]
matmul per core (batch data-parallel over 8 cores) fused with LayerNorm.

Precision scheme: LayerNorm (with no bias, unit gamma in this problem) is
scale-invariant, so arbitrary global scales on x and C are free.  The 4096-dim
contraction is split K = KA*128 (bf16) + KOB*128 (fp8e4 via DoubleRow, 2x PE
throughput).  fp8e4 elementwise RMS quant error ~2.4%; the dot-product error
contribution is sqrt(2*KOB/32)*2.4%, kept under the 2e-2 harness gate with
margin.
"""

import os
import sys

import numpy as np

if "/opt/trn_rl_repo" not in sys.path:
    sys.path.insert(0, "/opt/trn_rl_repo")

import ml_dtypes

P = 128              # SBUF partitions
DIM = 4096
B = 8192
NCORES = 8
BL = B // NCORES     # batch rows per core
BT = BL // P         # b tiles per core
KO = DIM // P        # contraction tiles (32)
JC = 512             # moving free-dim chunk (output cols per matmul)
NJC = DIM // JC      # 8
EPS = 1e-5

# K split: KA 128-chunks in bf16, KOB in fp8e4 (DoubleRow pairs). KOB even.
KOB = int(os.environ.get("KERNEL_KOB", "10"))
KA = KO - KOB
SX8 = 32.0           # fp8 scale for x  (|x| < 5.7  -> *32  < 240)
SC8 = 1024.0         # fp8 scale for C  (|C| < 0.16 -> *1024 < 240)

_BUILD_CACHE = {}


def _build(apply_bias: bool, apply_affine: bool):
    key = (apply_bias, apply_affine, KOB)
    if key in _BUILD_CACHE:
        return _BUILD_CACHE[key]

    import concourse.mybir as mybir
    import concourse.tile as tile
    from concourse.tile import add_dep_helper
    from concourse import bacc

    bf16 = mybir.dt.bfloat16
    f16 = mybir.dt.float16
    f8 = mybir.dt.float8e4
    f32 = mybir.dt.float32

    nc = bacc.Bacc("TRN2", target_bir_lowering=False, debug=False,
                   num_devices=NCORES)

    xta_d = nc.declare_dram_parameter("xta", [BT, P, KA, P], bf16,
                                      isOutput=False)
    cta_d = nc.declare_dram_parameter("cta", [NJC, P, KA, JC], bf16,
                                      isOutput=False)
    if KOB:
        xtb_d = nc.declare_dram_parameter("xtb", [BT, P, KOB, P], f8,
                                          isOutput=False)
        ctb_d = nc.declare_dram_parameter("ctb", [NJC, P, KOB, JC], f8,
                                          isOutput=False)
    bias_d = nc.declare_dram_parameter("bias", [DIM], f32, isOutput=False)
    gamma_d = nc.declare_dram_parameter("gamma", [DIM], f32, isOutput=False)
    beta_d = nc.declare_dram_parameter("beta", [DIM], f32, isOutput=False)
    # fp16 output (upcast on host): halves the output traffic; 10 mantissa
    # bits is plenty for LayerNorm-scale values.
    out_d = nc.declare_dram_parameter("out", [BT, P, NJC, JC], f16,
                                      isOutput=True)

    # Non-bias builds apply the last two chunks straight from PSUM, so y_sb
    # holds NJC-2 chunks and the freed SBUF buys a third cta stream buffer
    # (which lets chunks 6/7 prefetch early enough for the bt-major tail).
    YC = NJC if apply_bias else NJC - 2
    WBUFS = 2 if apply_bias else 3

    with tile.TileContext(nc) as tc:
        with tc.tile_pool(name="xpool", bufs=1) as xpool, \
             tc.tile_pool(name="wpool", bufs=WBUFS) as wpool, \
             tc.tile_pool(name="ypool", bufs=1) as ypool, \
             tc.tile_pool(name="spool", bufs=1) as spool, \
             tc.tile_pool(name="opool", bufs=3) as opool, \
             tc.tile_pool(name="small", bufs=4) as small, \
             tc.tile_pool(name="ppool", bufs=4, space="PSUM") as ppool:

            xta_sb = xpool.tile([P, BT, KA, P], bf16, name="xta_sb")
            xtb_sb = (xpool.tile([P, BT, KOB, P], f8, name="xtb_sb")
                      if KOB else None)

            # PE warmup: independent matmuls on a memset scratch tile run
            # during the initial DMA head (no data deps), so the HAM clock
            # gate reaches 2.4 GHz before the real matmuls start.  ~16 cold
            # MMs cover the 3.4us SHORT window; the first chunk's data lands
            # at ~8-10us, so don't queue more than that (warmups would block
            # the real MMs behind them in the FIFO engine queue).
            warm_sb = small.tile([P, 384], bf16)
            nc.gpsimd.memset(warm_sb, 0.0)
            warm_ps = ppool.tile([P, 256], f32, name="warm_ps", tag="ps")
            for _ in range(45):
                nc.tensor.matmul(warm_ps, lhsT=warm_sb[:, 0:P],
                                 rhs=warm_sb[:, P:P + 256],
                                 start=True, stop=True)

            # Prefetch chunk 0 (both dtypes) split finely so the first
            # accumulation group's dependencies resolve early, then x b-tile
            # 0, chunk 1, then the remaining x b-tiles.
            ct_tiles = {}
            ct_tiles[0] = wpool.tile([P, KA, JC], bf16, name="cta_sb",
                                     tag="cta")
            ctb_tiles = {}
            if KOB:
                ctb_tiles[0] = wpool.tile([P, KOB, JC], f8, name="ctb_sb",
                                          tag="ctb")
            # HEAD: all hardware-dynamic DMA queues share one pool of 16
            # DMA engines (~330 GB/s aggregate), and each queue keeps ~6
            # instructions in flight, so bulk prefetch packets would
            # interleave with (and starve) the critical first-phase data.
            # Critical set = cta c0 + xta bt0 + fp8 c0/bt0 (~4 MiB), split
            # into ko-ordered pieces across all three rings; every other
            # input DMA is dependency-gated on the tail of the critical set.
            rings = [nc.sync, nc.scalar, nc.gpsimd]
            crit = []
            # x b-tile 0 first on each ring (consumed from ko=0), then the
            # c0 pieces in ko order round-robin, fp8 bits on gpsimd.
            h = KA // 2
            crit.append(nc.sync.dma_start(out=xta_sb[:, 0, :h],
                                          in_=xta_d[0, :, :h]))
            crit.append(nc.scalar.dma_start(out=xta_sb[:, 0, h:],
                                            in_=xta_d[0, :, h:]))
            if KOB:
                crit.append(nc.gpsimd.dma_start(out=xtb_sb[:, 0],
                                                in_=xtb_d[0]))
                crit.append(nc.gpsimd.dma_start(out=ctb_tiles[0],
                                                in_=ctb_d[0]))
            cuts = list(range(0, KA - 2, 3))
            for i, (a, b) in enumerate(zip(cuts, cuts[1:] + [KA])):
                crit.append(rings[i % 3].dma_start(out=ct_tiles[0][:, a:b],
                                                   in_=cta_d[0, :, a:b]))
            gate = crit[-1].ins

            def gated(eng, out, in_):
                d = eng.dma_start(out=out, in_=in_)
                add_dep_helper(d.ins, gate,
                               reason="bulk input DMA after critical head")

            # Ungated second tier: x b-tiles 1-3 and chunk 1 — needed inside
            # the first two (half-)phases, so they queue right behind the
            # critical set.  Everything later is gated.
            ct_tiles[1] = wpool.tile([P, KA, JC], bf16, name="cta_sb",
                                     tag="cta")
            if KOB:
                ctb_tiles[1] = wpool.tile([P, KOB, JC], f8, name="ctb_sb",
                                          tag="ctb")
            for bt in range(1, 4):
                rings[bt % 3].dma_start(out=xta_sb[:, bt], in_=xta_d[bt])
                if KOB:
                    rings[(bt + 1) % 3].dma_start(out=xtb_sb[:, bt],
                                                  in_=xtb_d[bt])
            nc.sync.dma_start(out=ct_tiles[1], in_=cta_d[1])
            if KOB:
                nc.gpsimd.dma_start(out=ctb_tiles[1], in_=ctb_d[1])
            for bt in range(4, 6):
                rings[bt % 3].dma_start(out=xta_sb[:, bt], in_=xta_d[bt])
                if KOB:
                    rings[(bt + 1) % 3].dma_start(out=xtb_sb[:, bt],
                                                  in_=xtb_d[bt])
            for bt in range(6, BT):
                gated(rings[bt % 3], xta_sb[:, bt], xta_d[bt])
                if KOB:
                    gated(rings[(bt + 1) % 3], xtb_sb[:, bt], xtb_d[bt])

            # y (pre-norm matmul result) stays resident in bf16.
            y_sb = ypool.tile([P, BT, YC, JC], bf16)
            # Per-chunk bn_stats, aggregated per b-tile at the end.
            stats_sb = spool.tile([P, BT, NJC, 6], f32)

            eps_sb = small.tile([P, 1], f32)
            nc.vector.memset(eps_sb, EPS)

            bias_sb = None
            if apply_bias:
                bias_sb = spool.tile([P, NJC, JC], f32)
                nc.sync.dma_start(out=bias_sb,
                                  in_=bias_d.ap().to_broadcast([P, NJC, JC]))

            gamma_sb = beta_sb = None
            if apply_affine:
                gamma_sb = spool.tile([P, NJC, JC], f32)
                nc.sync.dma_start(out=gamma_sb,
                                  in_=gamma_d.ap().to_broadcast([P, NJC, JC]))
                beta_sb = spool.tile([P, NJC, JC], f32)
                nc.sync.dma_start(out=beta_sb,
                                  in_=beta_d.ap().to_broadcast([P, NJC, JC]))

            JL = NJC // 2  # two LayerNorm-apply chunks of 2048 columns

            def layernorm_apply(bt, ps6=None, ps7=None):
                """Aggregate stats and write the normalized b-tile.

                The apply is (y - mu) * rstd, computed as two half-tiles:
                one on the Scalar engine (Identity(y*rstd + (-mu*rstd)) with
                per-partition scale/bias) and one on DVE (tensor_scalar), so
                each engine's per-b-tile work stays under the PE shadow of
                the remaining matmuls.  Chunks NJC-2/NJC-1 never went
                through PSUM->SBUF eviction; they're normalized straight out
                of PSUM (ps6/ps7).  Out-DMAs alternate between the gpsimd
                and sync rings.
                """
                mv = small.tile([P, 2], f32)
                nc.vector.bn_aggr(mv, stats_sb[:, bt, :, :])
                std = small.tile([P, 1], f32)
                nc.scalar.activation(std, mv[:, 1:2],
                                     mybir.ActivationFunctionType.Sqrt,
                                     bias=eps_sb)
                rstd = small.tile([P, 1], f32)
                nc.vector.reciprocal(rstd, std)
                nmr = small.tile([P, 1], f32)
                nc.vector.tensor_scalar(
                    nmr, mv[:, 0:1], scalar1=rstd, scalar2=-1.0,
                    op0=mybir.AluOpType.mult, op1=mybir.AluOpType.mult,
                )

                def dve_norm(dst, src):
                    nc.vector.tensor_scalar(
                        dst, src, scalar1=mv[:, 0:1], scalar2=rstd,
                        op0=mybir.AluOpType.subtract,
                        op1=mybir.AluOpType.mult,
                    )

                # Output DMAs are split 2 (+3 for the PSUM chunks) per
                # b-tile half and spread over the gpsimd+sync rings so the
                # final b-tile's 1 MiB drains in parallel pieces instead of
                # two serial 512 KiB transfers.
                orings = [nc.gpsimd, nc.sync]
                for i, j0 in enumerate(range(0, NJC, JL)):
                    o = opool.tile([P, JL, JC], f16)
                    pieces = [(0, JL)]
                    if i == 0:
                        # Two ACT instructions so the first piece's output
                        # DMA can launch ~1us earlier.
                        nc.scalar.activation(
                            o[:, 0:2, :], y_sb[:, bt, j0:j0 + 2, :],
                            mybir.ActivationFunctionType.Identity,
                            bias=nmr, scale=rstd,
                        )
                        nc.scalar.activation(
                            o[:, 2:JL, :], y_sb[:, bt, j0 + 2:j0 + JL, :],
                            mybir.ActivationFunctionType.Identity,
                            bias=nmr, scale=rstd,
                        )
                        pieces = [(0, 2), (2, JL)]
                    elif ps6 is not None:
                        dve_norm(o[:, :JL - 2, :],
                                 y_sb[:, bt, j0:j0 + JL - 2, :])
                        dve_norm(o[:, JL - 2, :], ps6)
                        dve_norm(o[:, JL - 1, :], ps7)
                        pieces = [(0, JL - 2), (JL - 2, JL - 1), (JL - 1, JL)]
                    else:
                        dve_norm(o, y_sb[:, bt, j0:j0 + JL, :])
                    if apply_affine:
                        nc.vector.tensor_mul(o, o, gamma_sb[:, j0:j0 + JL, :])
                        nc.vector.tensor_add(o, o, beta_sb[:, j0:j0 + JL, :])
                        pieces = [(0, JL)]
                    for k, (a, b) in enumerate(pieces):
                        eng = orings[(bt + i + k) % 2]
                        eng.dma_start(out=out_d[bt, :, j0 + a:j0 + b, :],
                                      in_=o[:, a:b, :])

            def matmul_group(ct_sb, ctb_sb, jc, bt):
                ps = ppool.tile([P, JC], f32, name="ps", tag="ps")
                # The fp8 DoubleRow MMs run as one streak after the bf16
                # MMs: measured cadence 28, 403, 216, 216, ... — only the
                # second DR pays an exposed weight load, so a streak is at
                # the ideal 2x DR rate (interleaving with bf16 is worse:
                # every bf16-after-DR then pays ~445ns).
                for ko in range(KA):
                    nc.tensor.matmul(
                        ps,
                        lhsT=xta_sb[:, bt, ko, :],
                        rhs=ct_sb[:, ko, :],
                        start=(ko == 0),
                        stop=(ko == KA - 1 and KOB == 0),
                    )
                for kp in range(0, KOB, 2):
                    nc.tensor.matmul(
                        ps,
                        lhsT=xtb_sb[:, bt, kp:kp + 2, :],
                        rhs=ctb_sb[:, kp:kp + 2, :],
                        start=False,
                        stop=(kp == KOB - 2),
                        perf_mode=mybir.MatmulPerfMode.DoubleRow,
                    )
                if apply_bias:
                    nc.vector.tensor_add(y_sb[:, bt, jc, :], ps,
                                         bias_sb[:, jc, :])
                    nc.vector.bn_stats(stats_sb[:, bt, jc, :],
                                       y_sb[:, bt, jc, :])
                else:
                    # DVE reads PSUM directly for the LayerNorm statistics;
                    # ACT evicts PSUM (cast to bf16) except for the last two
                    # chunks, which the epilogue normalizes out of PSUM.
                    if jc < YC:
                        nc.scalar.activation(
                            y_sb[:, bt, jc, :], ps,
                            mybir.ActivationFunctionType.Copy)
                    nc.vector.bn_stats(stats_sb[:, bt, jc, :], ps)
                return ps

            def prefetch(c):
                ct_tiles[c] = wpool.tile([P, KA, JC], bf16,
                                         name="cta_sb", tag="cta")
                nc.sync.dma_start(out=ct_tiles[c], in_=cta_d[c])
                if KOB:
                    ctb_tiles[c] = wpool.tile([P, KOB, JC], f8,
                                              name="ctb_sb", tag="ctb")
                    nc.gpsimd.dma_start(out=ctb_tiles[c], in_=ctb_d[c])

            # Phases 0..NJC-3: jc-major (each chunk streams through all
            # b-tiles).  Last two phases: bt-major, so b-tile completions
            # (LayerNorm apply + 1 MiB output DMA each) spread over ~100us
            # of matmul shadow instead of piling up behind the last phase.
            # The first two chunks run as HALF-phases (4 b-tiles each) so
            # the start-of-kernel DMA demand (chunk + its b-tiles' x) is
            # halved: the head becomes compute-bound at ~10us instead of
            # data-walking for ~30us.  Chunk c is prefetched at phase
            # index c (its wpool slot is free by then; lands 2 phases
            # before use).
            sched = [(0, range(0, 4)), (1, range(0, 4)),
                     (0, range(4, 8)), (1, range(4, 8))]
            sched += [(jc, range(BT)) for jc in range(2, NJC - 2)]
            for pi, (jc, bts) in enumerate(sched):
                ct_sb = ct_tiles[jc]
                ctb_sb = ctb_tiles[jc] if KOB else None
                for gi, bt in enumerate(bts):
                    if gi == 1 and 2 <= pi < NJC:
                        prefetch(pi)
                    matmul_group(ct_sb, ctb_sb, jc, bt)
            c6, c7 = ct_tiles.pop(NJC - 2), ct_tiles.pop(NJC - 1)
            cb6 = ctb_tiles.pop(NJC - 2) if KOB else None
            cb7 = ctb_tiles.pop(NJC - 1) if KOB else None
            for bt in range(BT):
                ps6 = matmul_group(c6, cb6, NJC - 2, bt)
                ps7 = matmul_group(c7, cb7, NJC - 1, bt)
                if apply_bias:
                    layernorm_apply(bt)
                else:
                    layernorm_apply(bt, ps6=ps6, ps7=ps7)

    nc.compile()
    _BUILD_CACHE[key] = nc
    return nc


def kernel(x, W_qkv, b_qkv, W_proj, b_proj, gamma, beta):
    from concourse.bass_utils import run_bass_kernel_spmd

    x = np.asarray(x, dtype=np.float32)
    W_qkv = np.asarray(W_qkv, dtype=np.float32)
    b_qkv = np.asarray(b_qkv, dtype=np.float32)
    W_proj = np.asarray(W_proj, dtype=np.float32)
    b_proj = np.asarray(b_proj, dtype=np.float32)
    gamma = np.asarray(gamma, dtype=np.float32)
    beta = np.asarray(beta, dtype=np.float32)

    # Fold the two projections (q/k are dead: seq len 1 => attention == v).
    W_v = W_qkv[2 * DIM:3 * DIM, :]
    C = W_proj @ W_v                          # [j, k]
    bias_total = W_proj @ b_qkv[2 * DIM:] + b_proj
    if KOB:
        bias_total = bias_total * (SX8 * SC8)

    # C^T tiled for streaming: ct[jc, p, ko, jl] = C[jc*JC+jl, ko*P+p]
    # With a hybrid split, the fp8 partial sums carry the SX8*SC8 = 2^15
    # scale, so the bf16 part must carry it too (exact in bf16: power of 2).
    # LayerNorm is scale-invariant, so the scale never needs removing.
    scale_tot = SX8 * SC8 if KOB else 1.0
    CtT = C.T.reshape(KO, P, NJC, JC).transpose(2, 1, 0, 3)  # [NJC,P,KO,JC]
    Cta = np.ascontiguousarray(CtT[:, :, :KA] * scale_tot).astype(
        ml_dtypes.bfloat16)
    if KOB:
        Ctb = np.ascontiguousarray(
            np.clip(CtT[:, :, KA:] * SC8, -240, 240)).astype(
            ml_dtypes.float8_e4m3)

    apply_bias = bool(np.any(bias_total))
    apply_affine = not (np.all(gamma == 1.0) and np.all(beta == 0.0))

    nc = _build(apply_bias, apply_affine)

    in_maps = []
    for i in range(NCORES):
        xs = x[i * BL:(i + 1) * BL]           # [BL, DIM]
        # xt[bt, p, ko, b'] = xs[bt*P + b', ko*P + p]
        xt = xs.T.reshape(KO, P, BT, P).transpose(2, 1, 0, 3)  # [BT,P,KO,P]
        m = {
            "xta": np.ascontiguousarray(
                xt[:, :, :KA]).astype(ml_dtypes.bfloat16),
            "cta": Cta,
            "bias": bias_total,
            "gamma": gamma,
            "beta": beta,
        }
        if KOB:
            m["xtb"] = np.ascontiguousarray(
                np.clip(xt[:, :, KA:] * SX8, -240, 240)).astype(
                ml_dtypes.float8_e4m3)
            m["ctb"] = Ctb
        in_maps.append(m)

    trace = bool(int(os.environ.get("KERNEL_TRACE", "0")))
    res = run_bass_kernel_spmd(nc, in_maps, core_ids=list(range(NCORES)),
                               trace=trace)
    if trace:
        kernel.last_exec_time_ns = res.exec_time_ns
        kernel.last_results = res

    out = np.concatenate(
        [r["out"].reshape(BL, DIM).astype(np.float32) for r in res.results],
        axis=0,
    )
    return out
